# revision 6
# baseline (speedup 1.0000x reference)
"""CRF decoder loss kernel for Trainium2 (8 NeuronCores, data-parallel over batch).

Algorithm (mathematically identical to the reference):
  The reference computes mean_b(Zp - score) where Zp is the CRF partition
  function of log_softmax(enc@W+b) and score is the gold-path score. Writing
  logits = R - logZ (R the raw projection scores, logZ the log-softmax
  normalizer), the normalizer cancels between Zp and score, so no softmax is
  ever needed. With a constant shift kappa for range control, the forward
  recursion runs in LINEAR space:

      P_0 = exp(start) * G_0,     P_t = (P_{t-1} @ exp(T)) * G_t,
      G_t = exp(R_t - kappa)                                  (all [B, V])

  loss_b = log(sum_j P_{len_b-1}[b,j] * exp(end_j))           <- S, device
           - sum_{t<len_b} (R[t,b,tgt_{t,b}] - kappa)         <- host (tiny)
           - (start[tgt_0] + sum T[tgt,tgt'] + end[tgt_last]) <- host (tiny)

Device work per core (batch shard of 32, v-major layouts):
  - scan: the 32-batch shard is split into two 16-batch groups that run the
    recursion STAGGERED: while group A's elementwise multiply runs on the
    vector engine, group B's four matmuls run on the PE (and vice versa), so
    the PE -> (DVE|ACT) -> PE dependency chain of one group is hidden under
    the other group's work. Per group-step the four 128x128 E-block matmuls
    accumulate into a single PSUM bank (per-element has_written semantics:
    the first matmul's start=True clears the whole bank, the jh=1 pair then
    overwrites its untouched columns), and ONE tensor_tensor [128,2,16]
    applies G_t and evicts to the bf16 ring. Group A's multiply runs on the
    vector engine, group B's on the scalar engine (separate PSUM read ports).
  - projection: R^T = W^T @ encT in FD-256 matmuls interleaved one-per-scan-
    step (instead of bursts) to fill PE gaps; ACT evicts G^T = exp(R^T+b-k).
  - S extraction: every 16 steps a batched matmul with exp(end) over the ring
    yields S_t[b] for all (t, b); host picks t = len_b - 1.
"""

import numpy as np
import ml_dtypes

import concourse.bacc as bacc
import concourse.tile as tile
from concourse import mybir
from concourse.bass_utils import run_bass_kernel_spmd

bf16 = ml_dtypes.bfloat16
f32 = mybir.dt.float32
bf16_t = mybir.dt.bfloat16

S, B, H, V = 512, 256, 512, 256
NCORES = 8
BC = B // NCORES            # 32 batch per core
ROWS = S * BC               # 16384 rows (t-major, b-minor)
KAPPA = 6.05
CHUNK = 512                 # projection chunk (rows) = 16 steps * 32 batch
NCHUNK = ROWS // CHUNK      # 32
NG = 2                      # scan batch groups per core (staggered)
GB = BC // NG               # 16
SBLK = 16                   # scan steps per S-extraction block
RING = 32                   # state ring slots

_nc_cache = None


def _build():
    nc = bacc.Bacc("TRN2", debug=False)

    encT = nc.dram_tensor("encT", [128, NCHUNK, 4, CHUNK], bf16_t, kind="ExternalInput")
    wblk = nc.dram_tensor("wblk", [128, 8, 128], bf16_t, kind="ExternalInput")
    expTblk = nc.dram_tensor("expTblk", [128, 4, 128], bf16_t, kind="ExternalInput")
    biasT = nc.dram_tensor("biasT", [128, 2], f32, kind="ExternalInput")
    expStartT = nc.dram_tensor("expStartT", [128, 2], f32, kind="ExternalInput")
    expEndT = nc.dram_tensor("expEndT", [128, 2], bf16_t, kind="ExternalInput")

    s_out = nc.dram_tensor("s_out", [1, ROWS], f32, kind="ExternalOutput")

    with tile.TileContext(nc) as tc:
        with (
            tc.tile_pool(name="consts", bufs=1) as consts,
            tc.tile_pool(name="encp", bufs=4) as encp,
            tc.tile_pool(name="gpool", bufs=1) as gpool,
            tc.tile_pool(name="proj_ps", bufs=2, space="PSUM") as proj_ps,
            tc.tile_pool(name="scan_ps", bufs=2, space="PSUM") as scan_ps,
            tc.tile_pool(name="s_ps", bufs=1, space="PSUM") as s_ps,
        ):
            w_sb = consts.tile([128, 8, 128], bf16_t)
            expT_sb = consts.tile([128, 4, 128], bf16_t)
            bias_sb = consts.tile([128, 2], f32)
            expStart_sb = consts.tile([128, 2], f32)
            expEnd_sb = consts.tile([128, 2], bf16_t)
            s_sb = consts.tile([1, ROWS], f32)
            ring = consts.tile([128, RING, 2, BC], bf16_t)

            nc.sync.dma_start(out=w_sb[:], in_=wblk[:])
            nc.sync.dma_start(out=expT_sb[:], in_=expTblk[:])
            nc.sync.dma_start(out=bias_sb[:], in_=biasT[:])
            nc.sync.dma_start(out=expStart_sb[:], in_=expStartT[:])
            nc.sync.dma_start(out=expEnd_sb[:], in_=expEndT[:])

            # ---------------- projection ----------------
            # chunk c covers scan steps [16c, 16c+16); R^T accumulated per
            # vocab-half into one PSUM bank as 8 FD-256 matmuls (4 H-blocks x
            # 2 column halves), then ACT evicts exp(R^T + b - kappa) as bf16.
            enc_tiles = {}
            gtiles = []
            pp = {}

            def emit_enc_dma(c):
                et = encp.tile([128, 4, CHUNK], bf16_t, name="et", tag="enc")
                nc.sync.dma_start(out=et[:], in_=encT[:, c, :, :])
                enc_tiles[c] = et

            def emit_proj_piece(c, i):
                # i in [0, 16): vh = i//8, cc = (i//4) % 2, ht = i%4
                vh, cc, ht = i // 8, (i // 4) % 2, i % 4
                if ht == 0 and cc == 0:
                    pp[(c, vh)] = proj_ps.tile([128, CHUNK], f32, name="pps",
                                               tag="pps")
                ps = pp[(c, vh)]
                et = enc_tiles[c]
                nc.tensor.matmul(
                    ps[:, cc * 256:(cc + 1) * 256],
                    lhsT=w_sb[:, ht * 2 + vh, :],
                    rhs=et[:, ht, cc * 256:(cc + 1) * 256],
                    start=(ht == 0 and cc == 0),
                    stop=(ht == 3 and cc == 1),
                )
                if ht == 3 and cc == 1:
                    g = gtiles[c]
                    nc.scalar.activation(
                        g[:, vh, :], ps[:],
                        mybir.ActivationFunctionType.Exp,
                        bias=bias_sb[:, vh:vh + 1], scale=1.0,
                    )
                    del pp[(c, vh)]
                    if vh == 1:
                        del enc_tiles[c]

            for c in range(NCHUNK):
                gtiles.append(gpool.tile([128, 2, CHUNK], bf16_t,
                                         name=f"g{c}", tag=f"g{c}"))

            def emit_sblock(k):
                # S_t for steps t in [k*SBLK, (k+1)*SBLK) from ring slots
                sp = s_ps.tile([1, SBLK * BC], f32, name="sps", tag="sps")
                s0 = (k * SBLK) % RING
                for ih in range(2):
                    nc.tensor.matmul(
                        sp[:],
                        lhsT=expEnd_sb[:, ih:ih + 1],
                        rhs=ring[:, s0:s0 + SBLK, ih, :],
                        start=(ih == 0),
                        stop=(ih == 1),
                    )
                nc.scalar.copy(
                    s_sb[0:1, k * (SBLK * BC):(k + 1) * (SBLK * BC)], sp[:])

            # ---------------- prologue ----------------
            for c in range(4):
                emit_enc_dma(c)
            for c in range(2):
                for i in range(16):
                    emit_proj_piece(c, i)

            for ih in range(2):
                nc.vector.tensor_scalar_mul(
                    ring[:, 0, ih, :],
                    in0=gtiles[0][:, ih, 0:BC],
                    scalar1=expStart_sb[:, ih:ih + 1],
                )

            # ---------------- scan ----------------
            for t in range(1, S):
                k = t // SBLK
                gt = gtiles[k]
                off = (t % SBLK) * BC
                for gi in range(NG):
                    b0 = gi * GB
                    ps = scan_ps.tile([128, 2, GB], f32,
                                      name=f"ps{gi}", tag=f"ps{gi}")
                    for jh in range(2):
                        for ih in range(2):
                            nc.tensor.matmul(
                                ps[:, jh, :],
                                lhsT=expT_sb[:, ih * 2 + jh, :],
                                rhs=ring[:, (t - 1) % RING, ih, b0:b0 + GB],
                                start=(jh == 0 and ih == 0),
                                stop=(jh == 1 and ih == 1),
                            )
                    nc.vector.tensor_tensor(
                        out=ring[:, t % RING, :, b0:b0 + GB],
                        in0=ps[:],
                        in1=gt[:, :, off + b0:off + b0 + GB],
                        op=mybir.AluOpType.mult,
                    )
                # interleaved projection: one FD-256 matmul per step keeps
                # the PE queue smooth; chunk c's 16 pieces span steps
                # 16(c-2)+1 .. 16(c-2)+16, one block ahead of its first use.
                i = (t - 1) % SBLK
                cp = (t - 1) // SBLK + 2
                if i == 0 and cp + 2 < NCHUNK:
                    emit_enc_dma(cp + 2)
                if cp < NCHUNK:
                    emit_proj_piece(cp, i)
                if t % SBLK == SBLK - 1:
                    emit_sblock(k)

            nc.sync.dma_start(out=s_out[:], in_=s_sb[:])

    nc.compile()
    return nc


def _host_consts(d):
    W_ = np.asarray(d["W"], dtype=np.float32)
    b_ = np.asarray(d["b"], dtype=np.float64)
    T_ = np.asarray(d["transition"], dtype=np.float64)
    start_ = np.asarray(d["start_transition"], dtype=np.float64)
    end_ = np.asarray(d["end_transition"], dtype=np.float64)
    Wb = np.ascontiguousarray(
        W_.reshape(4, 128, 2, 128).transpose(1, 0, 2, 3).reshape(128, 8, 128)
    ).astype(bf16)
    expTb = np.ascontiguousarray(
        np.exp(T_).reshape(2, 128, 2, 128).transpose(1, 0, 2, 3).reshape(128, 4, 128)
    ).astype(bf16)
    biasT = np.ascontiguousarray(
        (b_ - KAPPA).reshape(2, 128).T).astype(np.float32)
    expStartT = np.ascontiguousarray(
        np.exp(start_).reshape(2, 128).T).astype(np.float32)
    expEndT = np.ascontiguousarray(
        np.exp(end_).reshape(2, 128).T).astype(bf16)
    return Wb, expTb, biasT, expStartT, expEndT


def _prep_core_inputs(core, enc_bf, Wb, expTb, biasT, expStartT, expEndT):
    # encT layout [h%128, chunk, h//128, row-in-chunk]; rows are t*BC + b
    b0 = core * BC
    e = enc_bf[:, b0:b0 + BC, :].transpose(2, 0, 1).reshape(4, 128, NCHUNK, CHUNK)
    e = np.ascontiguousarray(e.transpose(1, 2, 0, 3))
    return {
        "encT": e, "wblk": Wb, "expTblk": expTb, "biasT": biasT,
        "expStartT": expStartT, "expEndT": expEndT,
    }


def kernel(enc_outs, W, b, transition, start_transition, end_transition,
           targets, lengths):
    global _nc_cache
    if _nc_cache is None:
        _nc_cache = _build()
    nc = _nc_cache

    enc = np.asarray(enc_outs, dtype=np.float32)
    W_ = np.asarray(W, dtype=np.float32)
    b_ = np.asarray(b, dtype=np.float64)
    T_ = np.asarray(transition, dtype=np.float64)
    start_ = np.asarray(start_transition, dtype=np.float64)
    end_ = np.asarray(end_transition, dtype=np.float64)
    tgt = np.asarray(targets).astype(np.int64)
    lens = np.asarray(lengths).astype(np.int64)

    Wb, expTb, biasT, expStartT, expEndT = _host_consts({
        "W": W, "b": b, "transition": transition,
        "start_transition": start_transition, "end_transition": end_transition,
    })
    enc_bf = enc.astype(bf16)
    in_maps = [
        _prep_core_inputs(c, enc_bf, Wb, expTb, biasT, expStartT, expEndT)
        for c in range(NCORES)
    ]
    res = run_bass_kernel_spmd(nc, in_maps, list(range(NCORES))).results

    # ---------------- host epilogue (small inputs only) ----------------
    tmask = (np.arange(S)[:, None] < lens[None, :])
    trans_sum = (T_[tgt[:-1], tgt[1:]] * tmask[1:]).sum(axis=0)
    last_tgt = tgt[lens - 1, np.arange(B)]
    hostscore = start_[tgt[0]] + trans_sum + end_[last_tgt]

    # gold-path raw emission scores: R[t, b, tgt] = enc[t, b] . W[:, tgt] + b
    # (16K dot products per core; 0.1% of the device FLOPs)
    Wg = W_.T[tgt.reshape(-1)]                        # (S*B, H)
    emis_all = (np.einsum("rh,rh->r", enc.reshape(S * B, H), Wg,
                          optimize=True).reshape(S, B)
                + b_[tgt])
    emis = ((emis_all - KAPPA) * tmask).sum(axis=0)

    loss_b = np.zeros(B, dtype=np.float64)
    for c in range(NCORES):
        b0 = c * BC
        s_flat = np.asarray(res[c]["s_out"], dtype=np.float64).reshape(ROWS)
        # S col layout: (t//SBLK) * 512 + (t%SBLK) * BC + b
        s_dec = s_flat.reshape(S // SBLK, SBLK, BC)
        bl = lens[b0:b0 + BC] - 1
        blocal = np.arange(BC)
        s_end = s_dec[bl // SBLK, bl % SBLK, blocal]
        loss_b[b0:b0 + BC] = np.log(s_end) - emis[b0:b0 + BC] \
            - hostscore[b0:b0 + BC]

    return np.float32(loss_b.mean())


# revision 8
# speedup vs baseline: 1.1273x; 1.1273x over previous
"""CRF decoder loss kernel for Trainium2 (8 NeuronCores, data-parallel over batch).

Algorithm (mathematically identical to the reference):
  The reference computes mean_b(Zp - score) where Zp is the CRF partition
  function of log_softmax(enc@W+b) and score is the gold-path score. Writing
  logits = R - logZ (R the raw projection scores, logZ the log-softmax
  normalizer), the normalizer cancels between Zp and score, so no softmax is
  ever needed. With a constant shift kappa for range control, the forward
  recursion runs in LINEAR space:

      P_0 = exp(start) * G_0,     P_t = (P_{t-1} @ exp(T)) * G_t,
      G_t = exp(R_t - kappa)                                  (all [B, V])

  loss_b = log(sum_j P_{len_b-1}[b,j] * exp(end_j))           <- S, device
           - sum_{t<len_b} (R[t,b,tgt_{t,b}] - kappa)         <- host (tiny)
           - (start[tgt_0] + sum T[tgt,tgt'] + end[tgt_last]) <- host (tiny)

Device work per core (batch shard of 32, v-major layouts).  The wall-clock is
the per-step dependency chain  PE matmuls -> sem -> DVE multiply -> sem -> PE,
so the kernel is organized to keep that chain minimal:
  - scan: per step four 128x128 E-block matmuls (fp32 PSUM, two banks psA/psB)
    and two DVE tensor_tensor ops (one per vocab half) that apply G_t and
    evict to the bf16 ring.  The first matmul's LDWEIGHTS is PREFETCHED with
    an explicit ldweights issued before the semaphore wait (the matmul itself
    carries ldweights=False), removing a ~150ns weight-load stall from the
    chain.
  - projection: R^T = W^T @ encT as FD-256 matmuls interleaved ONE PER STEP
    into the chain's PE slack (instead of 2.9us bursts every 16 steps); ACT
    evicts G^T = exp(R^T + b - kappa) as bf16 in step-major layout so the
    scan's TT reads are contiguous.
  - S extraction: every 16 steps a batched matmul with exp(end) over the ring
    yields S_t[b] for all (t, b); host picks t = len_b - 1.
"""

import numpy as np
import ml_dtypes

import concourse.bacc as bacc
import concourse.tile as tile
from concourse import mybir
from concourse.bass_utils import run_bass_kernel_spmd

bf16 = ml_dtypes.bfloat16
f32 = mybir.dt.float32
bf16_t = mybir.dt.bfloat16

S, B, H, V = 512, 256, 512, 256
NCORES = 8
BC = B // NCORES            # 32 batch per core
ROWS = S * BC               # 16384 rows (t-major, b-minor)
KAPPA = 6.05
CHUNK = 512                 # projection chunk (rows) = 16 steps * 32 batch
NCHUNK = ROWS // CHUNK      # 32
SBLK = 16                   # scan steps per S-extraction block
RING = 32                   # state ring slots

_nc_cache = None


def _build():
    nc = bacc.Bacc("TRN2", debug=False)

    encT = nc.dram_tensor("encT", [128, NCHUNK, 4, CHUNK], bf16_t, kind="ExternalInput")
    wblk = nc.dram_tensor("wblk", [128, 8, 128], bf16_t, kind="ExternalInput")
    expTblk = nc.dram_tensor("expTblk", [128, 4, 128], bf16_t, kind="ExternalInput")
    biasT = nc.dram_tensor("biasT", [128, 2], f32, kind="ExternalInput")
    expStartT = nc.dram_tensor("expStartT", [128, 2], f32, kind="ExternalInput")
    expEndT = nc.dram_tensor("expEndT", [128, 2], bf16_t, kind="ExternalInput")

    s_out = nc.dram_tensor("s_out", [1, ROWS], f32, kind="ExternalOutput")

    with tile.TileContext(nc) as tc:
        with (
            tc.tile_pool(name="consts", bufs=1) as consts,
            tc.tile_pool(name="encp", bufs=4) as encp,
            tc.tile_pool(name="gpool", bufs=1) as gpool,
            tc.tile_pool(name="proj_ps", bufs=2, space="PSUM") as proj_ps,
            tc.tile_pool(name="scan_ps", bufs=2, space="PSUM") as scan_ps,
            tc.tile_pool(name="s_ps", bufs=1, space="PSUM") as s_ps,
        ):
            w_sb = consts.tile([128, 8, 128], bf16_t)
            expT_sb = consts.tile([128, 4, 128], bf16_t)
            bias_sb = consts.tile([128, 2], f32)
            expStart_sb = consts.tile([128, 2], f32)
            expEnd_sb = consts.tile([128, 2], bf16_t)
            s_sb = consts.tile([1, ROWS], f32)
            ring = consts.tile([128, RING, 2, BC], bf16_t)

            nc.sync.dma_start(out=w_sb[:], in_=wblk[:])
            nc.sync.dma_start(out=expT_sb[:], in_=expTblk[:])
            nc.sync.dma_start(out=bias_sb[:], in_=biasT[:])
            nc.sync.dma_start(out=expStart_sb[:], in_=expStartT[:])
            nc.sync.dma_start(out=expEnd_sb[:], in_=expEndT[:])

            # ---------------- projection ----------------
            # chunk c covers scan steps [16c, 16c+16); per chunk 16 FD-256
            # matmuls (2 vocab halves x 2 column halves x 4 H blocks) feed two
            # one-bank PSUM tiles [128, 16, 32]; ACT evicts exp(R+b-kappa)
            # into the STEP-MAJOR g tile [128, 16, 2, 32] so scan TT reads
            # g[:, step, jh, :] contiguously.
            enc_tiles = {}
            gtiles = []
            pp = {}

            def emit_enc_dma(c):
                et = encp.tile([128, 4, CHUNK], bf16_t, name="et", tag="enc")
                nc.sync.dma_start(out=et[:], in_=encT[:, c, :, :])
                enc_tiles[c] = et

            def emit_proj_piece(c, i):
                # i in [0, 16): vh = i//8, cc = (i//4) % 2, ht = i%4
                vh, cc, ht = i // 8, (i // 4) % 2, i % 4
                if ht == 0 and cc == 0:
                    pp[(c, vh)] = proj_ps.tile([128, SBLK, BC], f32,
                                               name="pps", tag="pps")
                ps = pp[(c, vh)]
                et = enc_tiles[c]
                nc.tensor.matmul(
                    ps[:, cc * 8:(cc + 1) * 8, :],
                    lhsT=w_sb[:, ht * 2 + vh, :],
                    rhs=et[:, ht, cc * 256:(cc + 1) * 256],
                    start=(ht == 0 and cc == 0),
                    stop=(ht == 3 and cc == 1),
                )
                if ht == 3 and cc == 1:
                    g = gtiles[c]
                    nc.scalar.activation(
                        g[:, :, vh, :], ps[:],
                        mybir.ActivationFunctionType.Exp,
                        bias=bias_sb[:, vh:vh + 1], scale=1.0,
                    )
                    del pp[(c, vh)]
                    if vh == 1:
                        del enc_tiles[c]

            for c in range(NCHUNK):
                gtiles.append(gpool.tile([128, SBLK, 2, BC], bf16_t,
                                         name=f"g{c}", tag=f"g{c}"))

            def emit_sblock(k):
                # S_t for steps t in [k*SBLK, (k+1)*SBLK) from ring slots
                sp = s_ps.tile([1, SBLK * BC], f32, name="sps", tag="sps")
                s0 = (k * SBLK) % RING
                for ih in range(2):
                    nc.tensor.matmul(
                        sp[:],
                        lhsT=expEnd_sb[:, ih:ih + 1],
                        rhs=ring[:, s0:s0 + SBLK, ih, :],
                        start=(ih == 0),
                        stop=(ih == 1),
                    )
                nc.scalar.copy(
                    s_sb[0:1, k * (SBLK * BC):(k + 1) * (SBLK * BC)], sp[:])

            # ---------------- prologue ----------------
            for c in range(4):
                emit_enc_dma(c)
            for c in range(2):
                for i in range(16):
                    emit_proj_piece(c, i)

            for ih in range(2):
                nc.vector.tensor_scalar_mul(
                    ring[:, 0, ih, :],
                    in0=gtiles[0][:, 0, ih, :],
                    scalar1=expStart_sb[:, ih:ih + 1],
                )

            # prefetch the first E-block weight load for step 1
            nc.tensor.ldweights(expT_sb[:, 0, :])

            # ---------------- scan ----------------
            for t in range(1, S):
                k = t // SBLK
                gt = gtiles[k]
                st = t % SBLK
                psA = scan_ps.tile([128, BC], f32, name="psA", tag="psA")
                psB = scan_ps.tile([128, BC], f32, name="psB", tag="psB")
                for jh, ps in ((0, psA), (1, psB)):
                    for ih in range(2):
                        mm = nc.tensor.matmul(
                            ps[:],
                            lhsT=expT_sb[:, ih * 2 + jh, :],
                            rhs=ring[:, (t - 1) % RING, ih, :],
                            start=(ih == 0),
                            stop=(ih == 1),
                        )
                        if jh == 0 and ih == 0:
                            # weights were prefetched by the explicit
                            # ldweights below (previous iteration)
                            mm.ldweights = False
                    nc.vector.tensor_tensor(
                        out=ring[:, t % RING, jh, :],
                        in0=ps[:],
                        in1=gt[:, st, jh, :],
                        op=mybir.AluOpType.mult,
                    )
                # interleaved projection: one FD-256 matmul per step; chunk
                # c's 16 pieces span steps 16(c-2)+1 .. 16(c-2)+16.
                i = (t - 1) % SBLK
                cp = (t - 1) // SBLK + 2
                if i == 0 and cp + 2 < NCHUNK:
                    emit_enc_dma(cp + 2)
                if cp < NCHUNK:
                    emit_proj_piece(cp, i)
                if t % SBLK == SBLK - 1:
                    emit_sblock(k)
                if t + 1 < S:
                    # prefetch next step's first E-block load so it runs
                    # during the DVE wait instead of stalling the first matmul
                    nc.tensor.ldweights(expT_sb[:, 0, :])

            nc.sync.dma_start(out=s_out[:], in_=s_sb[:])

    nc.compile()
    return nc


def _host_consts(d):
    W_ = np.asarray(d["W"], dtype=np.float32)
    b_ = np.asarray(d["b"], dtype=np.float64)
    T_ = np.asarray(d["transition"], dtype=np.float64)
    start_ = np.asarray(d["start_transition"], dtype=np.float64)
    end_ = np.asarray(d["end_transition"], dtype=np.float64)
    Wb = np.ascontiguousarray(
        W_.reshape(4, 128, 2, 128).transpose(1, 0, 2, 3).reshape(128, 8, 128)
    ).astype(bf16)
    expTb = np.ascontiguousarray(
        np.exp(T_).reshape(2, 128, 2, 128).transpose(1, 0, 2, 3).reshape(128, 4, 128)
    ).astype(bf16)
    biasT = np.ascontiguousarray(
        (b_ - KAPPA).reshape(2, 128).T).astype(np.float32)
    expStartT = np.ascontiguousarray(
        np.exp(start_).reshape(2, 128).T).astype(np.float32)
    expEndT = np.ascontiguousarray(
        np.exp(end_).reshape(2, 128).T).astype(bf16)
    return Wb, expTb, biasT, expStartT, expEndT


def _prep_core_inputs(core, enc_bf, Wb, expTb, biasT, expStartT, expEndT):
    # encT layout [h%128, chunk, h//128, row-in-chunk]; rows are t*BC + b
    b0 = core * BC
    e = enc_bf[:, b0:b0 + BC, :].transpose(2, 0, 1).reshape(4, 128, NCHUNK, CHUNK)
    e = np.ascontiguousarray(e.transpose(1, 2, 0, 3))
    return {
        "encT": e, "wblk": Wb, "expTblk": expTb, "biasT": biasT,
        "expStartT": expStartT, "expEndT": expEndT,
    }


def kernel(enc_outs, W, b, transition, start_transition, end_transition,
           targets, lengths):
    global _nc_cache
    if _nc_cache is None:
        _nc_cache = _build()
    nc = _nc_cache

    enc = np.asarray(enc_outs, dtype=np.float32)
    W_ = np.asarray(W, dtype=np.float32)
    b_ = np.asarray(b, dtype=np.float64)
    T_ = np.asarray(transition, dtype=np.float64)
    start_ = np.asarray(start_transition, dtype=np.float64)
    end_ = np.asarray(end_transition, dtype=np.float64)
    tgt = np.asarray(targets).astype(np.int64)
    lens = np.asarray(lengths).astype(np.int64)

    Wb, expTb, biasT, expStartT, expEndT = _host_consts({
        "W": W, "b": b, "transition": transition,
        "start_transition": start_transition, "end_transition": end_transition,
    })
    enc_bf = enc.astype(bf16)
    in_maps = [
        _prep_core_inputs(c, enc_bf, Wb, expTb, biasT, expStartT, expEndT)
        for c in range(NCORES)
    ]
    res = run_bass_kernel_spmd(nc, in_maps, list(range(NCORES))).results

    # ---------------- host epilogue (small inputs only) ----------------
    tmask = (np.arange(S)[:, None] < lens[None, :])
    trans_sum = (T_[tgt[:-1], tgt[1:]] * tmask[1:]).sum(axis=0)
    last_tgt = tgt[lens - 1, np.arange(B)]
    hostscore = start_[tgt[0]] + trans_sum + end_[last_tgt]

    # gold-path raw emission scores: R[t, b, tgt] = enc[t, b] . W[:, tgt] + b
    # (16K dot products per core; 0.1% of the device FLOPs)
    Wg = W_.T[tgt.reshape(-1)]                        # (S*B, H)
    emis_all = (np.einsum("rh,rh->r", enc.reshape(S * B, H), Wg,
                          optimize=True).reshape(S, B)
                + b_[tgt])
    emis = ((emis_all - KAPPA) * tmask).sum(axis=0)

    loss_b = np.zeros(B, dtype=np.float64)
    for c in range(NCORES):
        b0 = c * BC
        s_flat = np.asarray(res[c]["s_out"], dtype=np.float64).reshape(ROWS)
        # S col layout: (t//SBLK) * 512 + (t%SBLK) * BC + b
        s_dec = s_flat.reshape(S // SBLK, SBLK, BC)
        bl = lens[b0:b0 + BC] - 1
        blocal = np.arange(BC)
        s_end = s_dec[bl // SBLK, bl % SBLK, blocal]
        loss_b[b0:b0 + BC] = np.log(s_end) - emis[b0:b0 + BC] \
            - hostscore[b0:b0 + BC]

    return np.float32(loss_b.mean())
